# revision 9
# baseline (speedup 1.0000x reference)
"""Trainium2 Bass kernel for the KB criterion loss.

Math
----
reference:
    diff[b,i,j] = probs[b,j] - probs[b,i]
    loss = sum_ij mean_b (diff^2 * C[i,j]) / (n_pos + 1e-8),  n_pos = count(C > 0)

Expanding the square removes the [B,N,N] intermediate:

    total = sum_i S2_i r_i + sum_j S2_j c_j - 2 sum_b P_b^T C P_b
        with S2_j = sum_b P[b,j]^2, r_i = sum_j C_ij, c_j = sum_i C_ij
    loss  = (total / B) / (n_pos + 1e-8)

Sharding (8 cores)
------------------
Shard C by rows: core k owns rows S_k = [128k, 128k+128). P replicated.
Inputs are shipped TRANSPOSED (j on partitions) and column-rolled by 128k
so every core runs the same program; the contraction over j=1024 runs as
8 accumulating chunks of K=128, split into two DMA halves so the PE can
start on chunks 0-3 while 4-7 are still in flight.

Host packs one bf16 tile per half, pk[p, c, 0:257]:
    pk[p,c,0:128] = P[b, gj]^T   (gj = (128c + p + 128k) % 1024; col b)
    pk[p,c,128]   = 1.0          (ones column -> row sums r_i)
    pk[p,c,129:257] = C[S_k, gj]^T  (col i local)

Per-core pipeline (bf16 matmuls, fp32 PSUM), per half h:
  PE  mm1: M[i, 0:129]  += ct_c^T @ [pt|1]_c  -> M = (C P^T), M[:,128] = r
  DVE psq_h = pt_h^2
  PE  mm2: M2[i, 0:128] += ct_c^T @ psq_c     -> sum_ib M2 = sum_j S2_j c_j
  ACT sign(ct_h) accum -> npos; copy r PSUM->SBUF
  DVE pm    = sum_b pt_0 * M     (scalar_tensor_tensor accum)  -> col 2
      part1 = sum_b psq_0 * r    (tensor_scalar 4x, accum)     -> col 1
      t2    = sum over M2        (tensor_reduce)               -> col 0
  DMA out cols [128, 5] = [t2, part1, pm, npos_a, npos_b]

Host sums the 8x[128,5] partials (the scalar all-reduce) and finishes:
loss = (sum(t2 + part1 - 2 pm) / B) / (sum(npos) + 1e-8).
"""

import ml_dtypes
import numpy as np

import concourse.bass as bass
import concourse.tile as tile
from concourse import mybir
from concourse.bass_utils import run_bass_kernel_spmd

B = 128
N = 1024
NCORES = 8
SH = N // NCORES  # 128 rows of C per core
NCH = N // 128  # 8 contraction chunks
HCH = NCH // 2  # chunks per DMA half
F32 = mybir.dt.float32
BF16 = mybir.dt.bfloat16
USE_FP8 = True
DT = mybir.dt.float8e4 if USE_FP8 else BF16
NPDT = mybir.dt.np(DT)


def build_bass() -> bass.Bass:
    nc = bass.Bass()
    pk_d = nc.dram_tensor("pk", [128, NCH, 257], DT, kind="ExternalInput")
    o_d = nc.dram_tensor("out", [128, 4], F32, kind="ExternalOutput")

    with tile.TileContext(nc) as tc:
        with (
            tc.tile_pool(name="sb", bufs=1) as sb,
            tc.tile_pool(name="ps", bufs=1, space="PSUM") as ps,
        ):
            pk = sb.tile([128, NCH, 257], DT)
            psq = sb.tile([128, NCH, 128], DT)
            sgn = sb.tile([128, NCH, 128], DT)
            scr_a = sb.tile([128, 128], DT)
            scr_b = sb.tile([128, 128], DT)
            r_sb = sb.tile([128, 1], F32)
            cols = sb.tile([128, 4], F32)

            m_ps = ps.tile([128, 129], F32)
            m2_ps = ps.tile([128, 128], F32)

            # Single packed DMA: fewest descriptors (128 x 2056B), one issue,
            # one semaphore -- minimizes cross-core DMA straggler exposure.
            nc.sync.dma_start(out=pk, in_=pk_d[:, :, :])

            # DVE: psq = pt^2, split in halves so mm2 chunks unblock early
            nc.vector.tensor_mul(
                psq[:, 0:HCH, :], pk[:, 0:HCH, 0:128], pk[:, 0:HCH, 0:128]
            )
            nc.vector.tensor_mul(
                psq[:, HCH:NCH, :], pk[:, HCH:NCH, 0:128], pk[:, HCH:NCH, 0:128]
            )

            # PE: M = (C P^T | r), then M2 = C (P^2)^T
            for c in range(NCH):
                nc.tensor.matmul(
                    m_ps,
                    pk[:, c, 129:257],
                    pk[:, c, 0:129],
                    start=(c == 0),
                    stop=(c == NCH - 1),
                )
            for c in range(NCH):
                nc.tensor.matmul(
                    m2_ps,
                    pk[:, c, 129:257],
                    psq[:, c, :],
                    start=(c == 0),
                    stop=(c == NCH - 1),
                )

            # ACT: npos count, then PSUM->SBUF copy of r
            nc.scalar.activation(
                sgn,
                pk[:, :, 129:257],
                mybir.ActivationFunctionType.Sign,
                accum_out=cols[:, 3:4],
            )
            # r copy + the two m_ps-dependent DVE reductions run as soon as
            # mm1 stops, overlapping the mm2 group (high_priority biases the
            # tile scheduler to place them early).
            with tc.high_priority():
                nc.scalar.copy(r_sb, m_ps[:, 128:129])
                nc.vector.scalar_tensor_tensor(
                    out=scr_b,
                    in0=pk[:, 0, 0:128],
                    scalar=1.0,
                    in1=m_ps[:, 0:128],
                    op0=mybir.AluOpType.mult,
                    op1=mybir.AluOpType.mult,
                    accum_out=cols[:, 2:3],
                )
                nc.vector.tensor_scalar(
                    scr_a,
                    psq[:, 0, :],
                    r_sb,
                    None,
                    mybir.AluOpType.mult,
                    op1=mybir.AluOpType.add,
                    accum_out=cols[:, 1:2],
                )
            nc.vector.tensor_reduce(
                out=cols[:, 0:1],
                in_=m2_ps,
                axis=mybir.AxisListType.X,
                op=mybir.AluOpType.add,
            )

            nc.sync.dma_start(out=o_d[:, :], in_=cols)

    _split_multi_waits(nc)
    return nc


def _split_multi_waits(nc: bass.Bass):
    """This walrus build accepts only ONE sync-wait per instruction
    ("Too many sync wait commands"). Peel extras onto same-engine NOPs that
    each stall on a single semaphore — semantically identical."""
    for bb in nc.main_func.blocks:
        insts = bb.instructions
        i = 0
        while i < len(insts):
            ins = insts[i]
            si = getattr(ins, "sync_info", None)
            if si is not None and si.on_wait is not None and len(si.on_wait) > 1:
                waits = list(si.on_wait)
                nops = []
                for j, w in enumerate(waits[:-1]):
                    nop = mybir.InstNoOp(
                        name=f"{ins.name}-wsplit{j}",
                        sync_info=mybir.SyncInfo(on_wait=[w], on_update=[]),
                        bass_nofuse=True,
                        engine=ins.engine,
                    )
                    nc.register_instruction(nop, overwrite=True)
                    nops.append(nop)
                si.on_wait = [waits[-1]]
                insts[i:i] = nops
                i += len(nops)
            i += 1


_NC = None


def _get_nc() -> bass.Bass:
    global _NC
    if _NC is None:
        _NC = build_bass()
    return _NC


def make_in_maps(probs: np.ndarray, co_matrix: np.ndarray):
    P = np.ascontiguousarray(np.asarray(probs, dtype=np.float32))
    C = np.ascontiguousarray(np.asarray(co_matrix, dtype=np.float32))
    PT = P.T  # [N(j), B(b)]
    in_maps = []
    for k in range(NCORES):
        sh = SH * k
        ptr = np.roll(PT, -sh, axis=0).reshape(NCH, 128, B).transpose(1, 0, 2)
        ctr = (
            np.roll(C[sh : sh + SH, :].T, -sh, axis=0)
            .reshape(NCH, 128, SH)
            .transpose(1, 0, 2)
        )
        buf = np.empty((128, NCH, 257), dtype=NPDT)
        buf[:, :, 0:128] = ptr
        buf[:, :, 128] = 1.0
        buf[:, :, 129:257] = ctr
        in_maps.append({"pk": buf})
    return in_maps


def finish(outs: np.ndarray) -> np.ndarray:
    """outs: [NCORES, 128, 4] per-core columns (t2, part1, pm, npos)."""
    o = outs.astype(np.float64)
    total = o[:, :, 0].sum() + o[:, :, 1].sum() - 2.0 * o[:, :, 2].sum()
    npos = o[:, :, 3].sum()
    loss = (total / float(B)) / (npos + 1e-8)
    return np.array(loss, dtype=np.float32)


def kernel(probs: np.ndarray, co_matrix: np.ndarray) -> np.ndarray:
    nc = _get_nc()
    in_maps = make_in_maps(probs, co_matrix)
    res = run_bass_kernel_spmd(nc, in_maps, list(range(NCORES)))
    outs = np.stack([r["out"] for r in res.results])
    return finish(outs)
